# revision 3
# baseline (speedup 1.0000x reference)
"""Segment mean-pool (BERT lattice embedding) Trainium2 Bass kernel.

Full-input contract: kernel(hidden[64,512,768] f32, word_ids[64,512] i32,
num_tokens=400) -> [64,400,768] f32.

Strategy: data-parallel over batch across 8 NeuronCores (8 samples each).
Per sample b the ragged segment mean  out[t] = mean_{s: wid[s]==t} hidden[s]
is computed as a matmul on the PE array:

    A_T[s, c] = (word_ids[b, s] == perm(c))      one-hot, built on-device
    psum[c, :] = sum_j A_T[j-chunk].T @ hidden[b, j-chunk]
    out[t, h] = psum[c, h] * recip[b, t]         recip = 1/max(count,1)

Word-axis layout: perm(c) = 4*(c%100) + c//100, i.e. psum chunk m
(columns [100m, 100m+100)) holds words t = 4p + m on psum partition p.
All four chunks of a sample land in one om tile [100, 4, H] whose DMA to
out[b] is 12 KB/partition contiguous (one descriptor per partition, no
ragged 400-row tail, one output DMA instruction per sample instead of
four).  The permutation comes for free out of the gpsimd iota pattern
[[1,4],[4,100]].

All matmuls run in float32r (FP22-truncated fp32): full PE rate at even
N>=256, ~2e-4 relative error, and no dtype casts of the 100 MB activation
tensor. Per-word piece-count reciprocals are derived on host from the
128 KB word_ids index tensor — index-side preprocessing; all heavy data
stays on device.

DMA plan (the kernel is HBM-bound: 12.6 MB in + 9.8 MB out per core at a
~415 GB/s practical per-core ceiling = ~54 us of unavoidable streaming):
  - inputs are split across BOTH HWDGE rings (sync: samples 0-3, scalar:
    samples 4-7) so descriptor generation ramps twice as fast and the
    combined stream saturates HBM early;
  - outputs alternate rings (even samples -> scalar, odd -> sync) so the
    output-only drain phase at the end also runs on both rings;
  - sample 0's input is split per j-chunk so the first accumulation can
    start as soon as chunk 0 lands.
"""

import numpy as np

B, S, H, T = 64, 512, 768, 400
N_CORES = 8
B_LOC = B // N_CORES  # samples per core
P = 128
J = S // P  # contraction chunks per sample
N0 = 384  # h-chunk split: two equal psum banks, balances the scale engines
MW = 100  # words per psum chunk (T = 4 * MW, psum partition p holds t=4p+m)
NM = 4  # word chunks per sample

_CACHED = {}


def build_program():
    """Build + compile the single-core Bass program (same NEFF on all cores)."""
    import concourse.bass as bass  # noqa: F401
    import concourse.mybir as mybir
    import concourse.tile as tile
    from concourse import bacc

    nc = bacc.Bacc(
        "TRN2",
        target_bir_lowering=False,
        debug=False,
        enable_asserts=False,
        num_devices=N_CORES,
    )
    f32 = mybir.dt.float32
    f32r = mybir.dt.float32r

    # float32r == fp32 bit layout; the PE truncates to FP22 on read. Declaring
    # the whole hidden/one-hot path float32r satisfies walrus's fp32r-producer
    # rule without any casts or extra copies.
    hidden_t = nc.dram_tensor("hidden", [B_LOC, S, H], f32r, kind="ExternalInput").ap()
    # word_ids host-prearranged as [P, B_LOC, J] fp32 (values < 400 are exact):
    # wid_pbj[p, b, j] = word_ids[b, 128j+p], the per-partition scalar for
    # piece-chunk j. tensor_scalar(is_equal) requires fp32 operands.
    wid_t = nc.dram_tensor("word_ids_pbj", [P, B_LOC, J], f32, kind="ExternalInput").ap()
    # Host-computed 1/max(count,1): recip_pbm[p, b, m] = recip[b, 4p+m].
    recip_t = nc.dram_tensor("recip_pbm", [MW, B_LOC, NM], f32, kind="ExternalInput").ap()
    out_t = nc.dram_tensor("out", [B_LOC, T, H], f32, kind="ExternalOutput").ap()

    with tile.TileContext(nc) as tc:
        with tc.tile_pool(name="const", bufs=1) as const_pool, \
             tc.tile_pool(name="hidp", bufs=B_LOC) as hid_pool, \
             tc.tile_pool(name="aTp", bufs=3) as aT_pool, \
             tc.tile_pool(name="outp", bufs=5) as out_pool, \
             tc.tile_pool(name="psum", bufs=4, space="PSUM") as psum_pool:

            # iota_t[p, c] = 4*(c % 100) + c // 100 on every partition: chunk
            # m's columns carry words t = 4p + m in psum-partition order.
            iota_t = const_pool.tile([P, T], f32, name="iota_t")
            nc.gpsimd.iota(
                iota_t,
                pattern=[[1, NM], [NM, MW]],
                base=0,
                channel_multiplier=0,
                allow_small_or_imprecise_dtypes=True,
            )

            # Small index tensors ride at the head of the scalar ring; the
            # sync ring starts streaming sample 0 at the same time.
            wid_sb = const_pool.tile([P, B_LOC, J], f32, name="wid_sb")
            nc.scalar.dma_start(out=wid_sb, in_=wid_t)
            recip_sb = const_pool.tile([MW, B_LOC, NM], f32, name="recip_sb")
            nc.scalar.dma_start(out=recip_sb, in_=recip_t)

            # Prefetch the whole input shard up front (fits in SBUF), split
            # across both HWDGE rings so descriptor generation and the data
            # ramp run twice as fast: sync takes samples 0-3, scalar 4-7.
            hids = []
            for b in range(B_LOC):
                hid = hid_pool.tile([P, J, H], f32r, name=f"hid{b}", tag="hid")
                src = hidden_t[b].rearrange("(j p) h -> p j h", p=P)
                eng = nc.sync if b < B_LOC // 2 else nc.scalar
                if b == 0:
                    # First sample split per j-chunk so the first accumulation
                    # can start as soon as chunk 0 lands.
                    for j in range(J):
                        eng.dma_start(out=hid[:, j, :], in_=src[:, j, :])
                else:
                    eng.dma_start(out=hid, in_=src)
                hids.append(hid)

            for b in range(B_LOC):
                hid = hids[b]
                aT = aT_pool.tile([P, J, T], f32r, name="aT", tag="aT")
                for j in range(J):
                    nc.vector.tensor_scalar(
                        aT[:, j, :],
                        iota_t,
                        wid_sb[:, b, j : j + 1],
                        None,
                        op0=mybir.AluOpType.is_equal,
                    )
                om = out_pool.tile([MW, NM, H], f32, name="om", tag="om")
                for m in range(NM):
                    ps0 = psum_pool.tile([MW, N0], f32, name="ps0", tag="ps0")
                    ps1 = psum_pool.tile([MW, H - N0], f32, name="ps1", tag="ps1")
                    for j in range(J):
                        nc.tensor.matmul(
                            ps0,
                            aT[:, j, m * MW : (m + 1) * MW],
                            hid[:, j, 0:N0],
                            start=(j == 0),
                            stop=(j == J - 1),
                        )
                    for j in range(J):
                        nc.tensor.matmul(
                            ps1,
                            aT[:, j, m * MW : (m + 1) * MW],
                            hid[:, j, N0:H],
                            start=(j == 0),
                            stop=(j == J - 1),
                        )
                    rec = recip_sb[:, b, m : m + 1]
                    # out = psum * (1/count): ACT and DVE each take one half,
                    # both read PSUM directly.
                    nc.scalar.mul(om[:, m, 0:N0], ps0, rec)
                    nc.vector.tensor_scalar_mul(om[:, m, N0:H], ps1, rec)
                # One output DMA per sample: psum partition p of chunk m is
                # word t = 4p+m, so om[p] maps to out rows 4p..4p+3 — a
                # 12 KB/partition contiguous write. Alternate rings so the
                # final output-only drain phase uses both DGEs.
                dst = out_t[b].rearrange("(p m) h -> p m h", m=NM)
                eng = nc.scalar if b % 2 == 0 else nc.sync
                eng.dma_start(out=dst, in_=om)

    nc.compile()
    return nc


def _prep_in_maps(hidden, word_ids):
    hidden = np.ascontiguousarray(np.asarray(hidden), dtype=np.float32).reshape(B, S, H)
    wid = np.ascontiguousarray(np.asarray(word_ids), dtype=np.int32).reshape(B, S)

    # Per-word piece counts -> 1/max(count,1).
    counts = np.zeros((B, T), np.int64)
    rows = np.repeat(np.arange(B), S)
    np.add.at(counts, (rows, wid.reshape(-1)), 1)
    recip = (1.0 / np.maximum(counts, 1)).astype(np.float32)  # [B, T]

    in_maps = []
    for i in range(N_CORES):
        sl = slice(i * B_LOC, (i + 1) * B_LOC)
        hs = np.ascontiguousarray(hidden[sl])
        ws = wid[sl]
        # [B_LOC, S] -> [P, B_LOC, J]: wid_pbj[p, b, j] = wid[b, 128j+p]
        wpbj = np.ascontiguousarray(
            ws.reshape(B_LOC, J, P).transpose(2, 0, 1).astype(np.float32)
        )
        # recip_pbm[p, b, m] = recip[b, 4p+m]
        rpbm = np.ascontiguousarray(recip[sl].reshape(B_LOC, MW, NM).transpose(1, 0, 2))
        in_maps.append({"hidden": hs, "word_ids_pbj": wpbj, "recip_pbm": rpbm})
    return in_maps


def run(hidden, word_ids, trace=False, **trace_kwargs):
    from concourse import bass_utils

    if "nc" not in _CACHED:
        _CACHED["nc"] = build_program()
    nc = _CACHED["nc"]
    in_maps = _prep_in_maps(hidden, word_ids)
    res = bass_utils.run_bass_kernel_spmd(
        nc, in_maps, core_ids=list(range(N_CORES)), trace=trace, **trace_kwargs
    )
    out = np.concatenate([res.results[i]["out"] for i in range(N_CORES)], axis=0)
    return out.astype(np.float32, copy=False), res


def kernel(hidden, word_ids, num_tokens=None, **_unused):
    out, _ = run(hidden, word_ids, trace=False)
    return out


# revision 9
# speedup vs baseline: 1.0195x; 1.0195x over previous
"""Segment mean-pool (BERT lattice embedding) Trainium2 Bass kernel.

Full-input contract: kernel(hidden[64,512,768] f32, word_ids[64,512] i32,
num_tokens=400) -> [64,400,768] f32.

Strategy: data-parallel over batch across 8 NeuronCores (8 samples each).
Per sample b the ragged segment mean  out[t] = mean_{s: wid[s]==t} hidden[s]
is computed as a matmul on the PE array:

    A_T[s, c] = (word_ids[b, s] == perm(c))      one-hot, built on-device
    psum[c, :] = sum_j A_T[j-chunk].T @ hidden[b, j-chunk]
    out[t, h] = psum[c, h] * recip[b, t]         recip = 1/max(count,1)

Word-axis layout: perm(c) = 4*(c%100) + c//100, i.e. psum chunk m
(columns [100m, 100m+100)) holds words t = 4p + m on psum partition p.
All four chunks of a sample land in one om tile [100, 4, H] whose DMA to
out[b] is 12 KB/partition contiguous (one descriptor per partition, no
ragged 400-row tail, one output DMA instruction per sample instead of
four).  The permutation comes for free out of the gpsimd iota pattern
[[1,4],[4,100]].

Dtypes: all-bf16 matmuls. The one-hot lhsT is bf16 (0/1 exact) and the
hidden activations are cast f32 -> bf16 on the DVE right after landing
(~0.4 us per [128, 768] chunk).  This halves the LDWEIGHTS time, which
is what actually paces the PE (~210 ns/matmul for 4-byte weights vs
~160 ns of moving-operand streaming), and walrus rejects mixed
32-bit x 16-bit matmuls so the moving side must be bf16 too.  bf16
rounding of the activations costs ~2e-3 relative error against a 2e-2
gate.  Per-word piece-count reciprocals are derived on host from the
128 KB word_ids index tensor — index-side preprocessing; all heavy data
stays on device.

DMA plan (kernel is HBM-bound: 12.6 MB in + 9.8 MB out per core at a
~415 GB/s practical per-core ceiling = ~54 us of unavoidable streaming):
  - one merged aux tensor (word ids + reciprocals, 256 B/partition) at
    the head of the sync ring — NOT two tiny-packet transfers that would
    clog a ring for ~16 us while the other ring hogs the SDMA engines;
  - all hidden prefetches on the sync ring (sample 0 split per j-chunk so
    the first accumulation starts as soon as chunk 0 lands);
  - outputs on the scalar ring, one DMA per sample, except the LAST
    sample which is split per m-chunk so the final write is 0.3 MB
    issued right after its scale, not 1.2 MB serialized after the whole
    sample's compute.
"""

import numpy as np

B, S, H, T = 64, 512, 768, 400
N_CORES = 8
B_LOC = B // N_CORES  # samples per core
P = 128
J = S // P  # contraction chunks per sample
N0 = 384  # h-chunk split: two equal psum banks, balances the scale engines
MW = 100  # words per psum chunk (T = 4 * MW, psum partition p holds t=4p+m)
NM = 4  # word chunks per sample

_CACHED = {}


def build_program():
    """Build + compile the single-core Bass program (same NEFF on all cores)."""
    import concourse.bass as bass  # noqa: F401
    import concourse.mybir as mybir
    import concourse.tile as tile
    from concourse import bacc

    nc = bacc.Bacc(
        "TRN2",
        target_bir_lowering=False,
        debug=False,
        enable_asserts=False,
        num_devices=N_CORES,
    )
    f32 = mybir.dt.float32
    bf16 = mybir.dt.bfloat16

    hidden_t = nc.dram_tensor("hidden", [B_LOC, S, H], f32, kind="ExternalInput").ap()
    # aux[p, b, 0:4] = word_ids[b, 128j+p] (fp32; values < 400 exact), the
    # per-partition scalar for piece-chunk j.  aux[p, b, 4:8] (p < 100) =
    # 1/max(count,1) for word t = 4p + m.  One chunky DMA instead of two
    # 128-byte-descriptor trickles.
    aux_t = nc.dram_tensor("aux_pb", [P, B_LOC, 2 * NM], f32, kind="ExternalInput").ap()
    out_t = nc.dram_tensor("out", [B_LOC, T, H], f32, kind="ExternalOutput").ap()

    with tile.TileContext(nc) as tc:
        with tc.tile_pool(name="const", bufs=1) as const_pool, \
             tc.tile_pool(name="hidf", bufs=4) as hidf_pool, \
             tc.tile_pool(name="hidb", bufs=B_LOC) as hidb_pool, \
             tc.tile_pool(name="aTp", bufs=3) as aT_pool, \
             tc.tile_pool(name="outp", bufs=5) as out_pool, \
             tc.tile_pool(name="psum", bufs=4, space="PSUM") as psum_pool:

            aux_sb = const_pool.tile([P, B_LOC, 2 * NM], f32, name="aux_sb")
            nc.sync.dma_start(out=aux_sb, in_=aux_t)

            # iota_t[p, c] = 4*(c % 100) + c // 100 on every partition: chunk
            # m's columns carry words t = 4p + m in psum-partition order.
            iota_t = const_pool.tile([P, T], f32, name="iota_t")
            nc.gpsimd.iota(
                iota_t,
                pattern=[[1, NM], [NM, MW]],
                base=0,
                channel_multiplier=0,
                allow_small_or_imprecise_dtypes=True,
            )

            # Prefetch the whole input shard up front (fits in SBUF) on the
            # sync ring; the scalar ring is reserved for the output stream.
            hidfs = []
            for b in range(B_LOC):
                hidf = hidf_pool.tile([P, J, H], f32, name=f"hidf{b}", tag="hidf")
                src = hidden_t[b].rearrange("(j p) h -> p j h", p=P)
                if b == 0:
                    # First sample split per j-chunk so the first accumulation
                    # can start as soon as chunk 0 lands.
                    for j in range(J):
                        nc.sync.dma_start(out=hidf[:, j, :], in_=src[:, j, :])
                else:
                    nc.sync.dma_start(out=hidf, in_=src)
                hidfs.append(hidf)

            for b in range(B_LOC):
                hid = hidb_pool.tile([P, J, H], bf16, name=f"hid{b}", tag="hid")
                if b == 0:
                    for j in range(J):
                        nc.vector.tensor_copy(hid[:, j, :], hidfs[b][:, j, :])
                else:
                    nc.vector.tensor_copy(hid, hidfs[b])
                aT = aT_pool.tile([P, J, T], bf16, name="aT", tag="aT")
                for j in range(J):
                    nc.vector.tensor_scalar(
                        aT[:, j, :],
                        iota_t,
                        aux_sb[:, b, j : j + 1],
                        None,
                        op0=mybir.AluOpType.is_equal,
                    )
                om = out_pool.tile([MW, NM, H], f32, name="om", tag="om")
                dst = out_t[b].rearrange("(p m) h -> p m h", m=NM)
                for m in range(NM):
                    ps0 = psum_pool.tile([MW, N0], f32, name="ps0", tag="ps0")
                    ps1 = psum_pool.tile([MW, H - N0], f32, name="ps1", tag="ps1")
                    for j in range(J):
                        nc.tensor.matmul(
                            ps0,
                            aT[:, j, m * MW : (m + 1) * MW],
                            hid[:, j, 0:N0],
                            start=(j == 0),
                            stop=(j == J - 1),
                        )
                    for j in range(J):
                        nc.tensor.matmul(
                            ps1,
                            aT[:, j, m * MW : (m + 1) * MW],
                            hid[:, j, N0:H],
                            start=(j == 0),
                            stop=(j == J - 1),
                        )
                    rec = aux_sb[:MW, b, NM + m : NM + m + 1]
                    # out = psum * (1/count): both halves on ACT (reads PSUM
                    # directly); DVE is busy with the bf16 casts + one-hots.
                    nc.scalar.mul(om[:, m, 0:N0], ps0, rec)
                    nc.scalar.mul(om[:, m, N0:H], ps1, rec)
                    if b == B_LOC - 1:
                        # Last sample: stream each chunk as soon as it's
                        # scaled so the final write is small.
                        nc.scalar.dma_start(out=dst[:, m], in_=om[:, m])
                if b < B_LOC - 1:
                    # One output DMA per sample: psum partition p of chunk m
                    # is word t = 4p+m, so om[p] maps to out rows 4p..4p+3 —
                    # a 12 KB/partition contiguous write.
                    nc.scalar.dma_start(out=dst, in_=om)

    nc.compile()
    return nc


def _prep_in_maps(hidden, word_ids):
    hidden = np.ascontiguousarray(np.asarray(hidden), dtype=np.float32).reshape(B, S, H)
    wid = np.ascontiguousarray(np.asarray(word_ids), dtype=np.int32).reshape(B, S)

    # Per-word piece counts -> 1/max(count,1).
    counts = np.zeros((B, T), np.int64)
    rows = np.repeat(np.arange(B), S)
    np.add.at(counts, (rows, wid.reshape(-1)), 1)
    recip = (1.0 / np.maximum(counts, 1)).astype(np.float32)  # [B, T]

    in_maps = []
    for i in range(N_CORES):
        sl = slice(i * B_LOC, (i + 1) * B_LOC)
        hs = np.ascontiguousarray(hidden[sl])
        ws = wid[sl]
        aux = np.ones((P, B_LOC, 2 * NM), np.float32)
        # aux[p, b, j] = wid[b, 128j+p]
        aux[:, :, :NM] = ws.reshape(B_LOC, J, P).transpose(2, 0, 1)
        # aux[p, b, 4+m] = recip[b, 4p+m]  (p < 100)
        aux[:MW, :, NM:] = recip[sl].reshape(B_LOC, MW, NM).transpose(1, 0, 2)
        in_maps.append({"hidden": hs, "aux_pb": np.ascontiguousarray(aux)})
    return in_maps


def run(hidden, word_ids, trace=False, **trace_kwargs):
    from concourse import bass_utils

    if "nc" not in _CACHED:
        _CACHED["nc"] = build_program()
    nc = _CACHED["nc"]
    in_maps = _prep_in_maps(hidden, word_ids)
    res = bass_utils.run_bass_kernel_spmd(
        nc, in_maps, core_ids=list(range(N_CORES)), trace=trace, **trace_kwargs
    )
    out = np.concatenate([res.results[i]["out"] for i in range(N_CORES)], axis=0)
    return out.astype(np.float32, copy=False), res


def kernel(hidden, word_ids, num_tokens=None, **_unused):
    out, _ = run(hidden, word_ids, trace=False)
    return out
